# revision 14
# baseline (speedup 1.0000x reference)
"""Trainium2 Bass kernel for nn_AligningModel (mel/phoneme GLU encoders + soft attention).

Strategy:
  - Data-parallel over batch: 32 samples -> 8 cores x 4 slots, length-sorted so
    each slot's compile-time bound is tight (slot j holds sorted ranks 8j..8j+7).
  - Length specialization: loop bounds per slot come from the actual mel/phoneme
    lens (program is SPMD-shared, bounds are per-slot maxima).  Valid masking
    makes everything beyond a sample's len exactly zero, so rows between len and
    the slot bound come out correct automatically; rows beyond the slot bound are
    a broadcast of the shared padded-row value.
  - Channel-major layout [C,T] on-chip so the k=3 convs are plain matmuls.
  - float32r matmuls (full PE speed at N>=256); fp32r = fp32 rounded to 11
    mantissa bits, so bytes remain valid fp32; host pre-rounds DMA-fed operands.
  - Scale folding: sqrt(0.5)^b folded into g-path conv weights; softmax uses
    logits = 2*C^8*dots(z_mel, z_ph) - ph_sq (mel_sq dropped: softmax-invariant),
    no max-subtraction (|logits| < 1 for 0.02-scaled weights), and the phoneme
    -1e9 mask folded into the per-partition exp bias.
  - Z (softmax denominator) via ones-columns appended to the time-major ph
    encoding inside the context matmul.
"""

import os
import numpy as np

B = 32
N_CORES = 8
SPC = 4           # samples (slots) per core
T_MEL = 2000
MEL_D = 80
D = 256
C = float(np.sqrt(0.5))
C4 = 0.25         # C**4 exact
C8 = 0.0625       # C**8 exact

_prog_cache = {}


def _round_fp32r(a):
    """Round fp32 to the fp32r grid (11-bit mantissa, low 12 bits zero, RNE)."""
    u = np.ascontiguousarray(a, np.float32).view(np.uint32)
    base = u >> np.uint32(12)
    low = u & np.uint32(0xFFF)
    inc = (low > 0x800) | ((low == 0x800) & ((base & np.uint32(1)) == 1))
    return ((base + inc.astype(np.uint32)) << np.uint32(12)).view(np.float32)


def _chunks(total, cap):
    """Split `total` into <=cap chunks, each a multiple of 4 (fp32r dst rule).
    Prefer equal chunks >=256 (full fp32r speed); else greedy cap + remainder."""
    assert total % 4 == 0 and total > 0
    n = -(-total // cap)
    base = min(cap, ((total + n - 1) // n + 3) // 4 * 4)
    if base < 256:
        base = cap
    out = []
    off = 0
    while off < total:
        w = min(base, total - off)
        out.append((off, w))
        off += w
    return out


def _host_prep(mels, phonemes, mel_lens, phoneme_lens, embedding,
               mel_conv_w, mel_conv_b, ph_w, ph_b, mel_w, mel_b, S_pad):
    """Build the per-core input maps (numpy only). Returns (in_maps, flags,
    perm, L, SL) where perm[8*j + c] = original sample index in core c slot j."""
    f32 = np.float32
    SP2 = S_pad + 2

    order = np.argsort(-np.asarray(mel_lens), kind="stable")
    perm = np.asarray(order)
    L = tuple(int(mel_lens[perm[8 * j]]) for j in range(SPC))
    SL = tuple(int(max(phoneme_lens[perm[8 * j + c]] for c in range(8)))
               for j in range(SPC))

    w0 = _round_fp32r(np.ascontiguousarray(np.transpose(mel_conv_w, (2, 1, 0)).astype(f32)))

    def pack_w(w4):
        out = np.empty((4, 3, 2, 128, 512), f32)
        for b in range(4):
            w = np.transpose(w4[b], (2, 1, 0)).astype(f32)  # [k, i, o]
            w = w.reshape(3, 2, 128, 512)
            w[:, :, :, 256:] *= f32(C ** b)
            out[b] = w
        return out
    wm = _round_fp32r(pack_w(mel_w))
    wp = _round_fp32r(pack_w(ph_w))
    idc4 = np.eye(128, dtype=f32)

    has_b0 = bool(np.any(mel_conv_b))
    has_bm = bool(np.any(mel_b))
    has_bp = bool(np.any(ph_b))
    shared = {"w0": w0, "wm": wm, "wp": wp, "idc4": idc4,
              "onecol": np.ones((128, 4), f32)}
    if has_b0:
        shared["b0r"] = mel_conv_b.astype(f32).reshape(1, 256)
    if has_bm:
        shared["bmar"] = mel_b[:, :256].astype(f32)
        shared["bmg"] = np.ascontiguousarray(
            mel_b[:, 256:].astype(f32).reshape(4, 2, 128).transpose(2, 0, 1).reshape(128, 8))
    if has_bp:
        shared["bpar"] = ph_b[:, :256].astype(f32)
        shared["bpg"] = np.ascontiguousarray(
            ph_b[:, 256:].astype(f32).reshape(4, 2, 128).transpose(2, 0, 1).reshape(128, 8))

    ar = np.arange(T_MEL)
    ars = np.arange(S_pad)
    in_maps = []
    for c in range(N_CORES):
        idx = [int(perm[8 * j + c]) for j in range(SPC)]
        m = dict(shared)
        mcm = np.zeros((SPC, MEL_D, T_MEL + 2), f32)
        vm = np.zeros((SPC, T_MEL + 2), f32)
        zph = np.zeros((SPC, 2, 128, SP2), f32)
        vph = np.zeros((SPC, SP2), f32)
        mv = np.full((SPC, S_pad), -1e9, f32)
        for j, b in enumerate(idx):
            mcm[j, :, 1:T_MEL + 1] = _round_fp32r(np.asarray(mels[b], f32).T)
            vm[j, 1:T_MEL + 1] = (ar < int(mel_lens[b])).astype(f32)
            pl = int(phoneme_lens[b])
            ph_pad = np.concatenate([[0], np.asarray(phonemes[b], np.int64)])[:S_pad]
            e = embedding[ph_pad].astype(f32)
            valid = (ars[:len(e)] <= pl)
            e[~valid] = 0.0
            zph[j, :, :, 1:1 + len(e)] = _round_fp32r(e.T.reshape(2, 128, len(e)))
            vph[j, 1:1 + len(e)] = valid.astype(f32)
            mv[j, :len(e)][valid] = 0.0
        m["mels_cm"] = mcm
        m["valid_mel"] = vm
        m["zph0"] = zph
        m["valid_ph"] = vph
        m["mvec"] = mv
        in_maps.append(m)
    return in_maps, (has_b0, has_bm, has_bp), perm, L, SL


def _build_program(S_pad, L, SL, has_b0, has_bm, has_bp):
    from contextlib import ExitStack
    import concourse.bass as bass
    import concourse.bacc as bacc
    import concourse.tile as tile
    from concourse import mybir

    f32 = mybir.dt.float32
    f32r = mybir.dt.float32r
    AF = mybir.ActivationFunctionType
    ALU = mybir.AluOpType
    AX = mybir.AxisListType
    SP2 = S_pad + 2

    # per-slot compile-time bounds
    W = [min(T_MEL, -(-(L[j] + 2) // 4) * 4) for j in range(SPC)]       # mel conv cols
    Tb = [min(T_MEL, -(-(L[j] + 2) // 128) * 128) for j in range(SPC)]  # attn rows
    SW = [min(S_pad, -(-(SL[j] + 2) // 4) * 4) for j in range(SPC)]     # ph conv cols
    NSB = [min(S_pad // 128, -(-(SL[j] + 2) // 128)) for j in range(SPC)]
    mel_chunks = [_chunks(W[j], 500) for j in range(SPC)]
    ph_chunks = [_chunks(SW[j], 512) for j in range(SPC)]
    dot_chunks = [_chunks(Tb[j], 512) for j in range(SPC)]

    nc = bacc.Bacc()
    t_mcm = nc.dram_tensor("mels_cm", [SPC, MEL_D, T_MEL + 2], f32r, kind="ExternalInput")
    t_vm = nc.dram_tensor("valid_mel", [SPC, T_MEL + 2], f32, kind="ExternalInput")
    t_zph = nc.dram_tensor("zph0", [SPC, 2, 128, SP2], f32r, kind="ExternalInput")
    t_vph = nc.dram_tensor("valid_ph", [SPC, SP2], f32, kind="ExternalInput")
    t_mv = nc.dram_tensor("mvec", [SPC, S_pad], f32, kind="ExternalInput")
    t_w0 = nc.dram_tensor("w0", [3, MEL_D, 256], f32r, kind="ExternalInput")
    t_wm = nc.dram_tensor("wm", [4, 3, 2, 128, 512], f32r, kind="ExternalInput")
    t_wp = nc.dram_tensor("wp", [4, 3, 2, 128, 512], f32r, kind="ExternalInput")
    t_id = nc.dram_tensor("idc4", [128, 128], f32r, kind="ExternalInput")
    t_one = nc.dram_tensor("onecol", [128, 4], f32r, kind="ExternalInput")
    t_b0 = nc.dram_tensor("b0r", [1, 256], f32, kind="ExternalInput") if has_b0 else None
    t_bmar = nc.dram_tensor("bmar", [4, 256], f32, kind="ExternalInput") if has_bm else None
    t_bmg = nc.dram_tensor("bmg", [128, 8], f32, kind="ExternalInput") if has_bm else None
    t_bpar = nc.dram_tensor("bpar", [4, 256], f32, kind="ExternalInput") if has_bp else None
    t_bpg = nc.dram_tensor("bpg", [128, 8], f32, kind="ExternalInput") if has_bp else None
    t_pad = nc.dram_tensor("padrow", [SPC, 512], f32)
    t_out = nc.dram_tensor("out", [SPC, T_MEL, 512], f32, kind="ExternalOutput")

    def bcast(ap, parts):
        return bass.AP(tensor=ap.tensor, offset=ap.offset, ap=[[0, parts]] + list(ap.ap))

    with tile.TileContext(nc) as tc, ExitStack() as ctx:
        wconst = ctx.enter_context(tc.tile_pool(name="wconst", bufs=1))
        wblk = ctx.enter_context(tc.tile_pool(name="wblk", bufs=2))
        ypool = ctx.enter_context(tc.tile_pool(name="y", bufs=4))
        zpool = ctx.enter_context(tc.tile_pool(name="zph", bufs=4))
        vpool = ctx.enter_context(tc.tile_pool(name="vm", bufs=2))
        vppool = ctx.enter_context(tc.tile_pool(name="vph", bufs=2))
        mpool = ctx.enter_context(tc.tile_pool(name="mcm", bufs=2))
        ympool = ctx.enter_context(tc.tile_pool(name="ym", bufs=10))
        ymppool = ctx.enter_context(tc.tile_pool(name="ymp", bufs=4))
        sgpool = ctx.enter_context(tc.tile_pool(name="sig", bufs=6))
        epool = ctx.enter_context(tc.tile_pool(name="exp", bufs=S_pad // 128))
        ztpool = ctx.enter_context(tc.tile_pool(name="ztm", bufs=S_pad // 128 + 2))
        sqpool = ctx.enter_context(tc.tile_pool(name="sq", bufs=2))
        spool = ctx.enter_context(tc.tile_pool(name="small", bufs=2 * (S_pad // 128) + 8))
        opool = ctx.enter_context(tc.tile_pool(name="octx", bufs=4))
        ppsum = ctx.enter_context(tc.tile_pool(name="pconv", bufs=4, space="PSUM"))
        apsum = ctx.enter_context(tc.tile_pool(name="pattn", bufs=2, space="PSUM"))
        cpsum = ctx.enter_context(tc.tile_pool(name="pctx", bufs=1, space="PSUM"))
        tpsum = ctx.enter_context(tc.tile_pool(name="ptp", bufs=1, space="PSUM"))

        # ---- constants ----
        w0_t = wconst.tile([MEL_D, 3, 256], f32r, tag="w0")
        nc.sync.dma_start(out=w0_t[:], in_=t_w0[:].rearrange("k i o -> i k o"))
        id_t = wconst.tile([128, 128], f32r, tag="id")
        nc.sync.dma_start(out=id_t[:], in_=t_id[:])
        need_ones = has_b0 or has_bm or has_bp
        if need_ones:
            ones_t = wconst.tile([1, 512], f32, tag="ones")
            nc.vector.memset(ones_t[:], 1.0)
        if has_b0:
            b0_t = wconst.tile([1, 256], f32, tag="b0")
            nc.sync.dma_start(out=b0_t[:], in_=t_b0[:])
        if has_bm:
            bmar_t = wconst.tile([4, 256], f32, tag="bmar")
            nc.sync.dma_start(out=bmar_t[:], in_=t_bmar[:])
            bmg_t = wconst.tile([128, 8], f32, tag="bmg")
            nc.sync.dma_start(out=bmg_t[:], in_=t_bmg[:])
        if has_bp:
            bpar_t = wconst.tile([4, 256], f32, tag="bpar")
            nc.sync.dma_start(out=bpar_t[:], in_=t_bpar[:])
            bpg_t = wconst.tile([128, 8], f32, tag="bpg")
            nc.sync.dma_start(out=bpg_t[:], in_=t_bpg[:])

        def glu_block(y_tiles, ym_tag, ym_pool, width, chunks, wt,
                      bar_t, bg_t, blk, vb):
            """One GLU block, channel-major, in-place on y_tiles."""
            yms = {}
            for icb in range(2):
                for (off, n) in chunks:
                    ym = ym_pool.tile([128, width], f32r, tag=ym_tag, name=ym_tag)
                    nc.vector.tensor_mul(out=ym[:, :n + 2],
                                         in0=y_tiles[icb][:, off:off + n + 2].bitcast(f32),
                                         in1=vb[:, off:off + n + 2])
                    yms[(icb, off)] = ym
            for oco in range(2):
                pa = {}
                pg = {}
                for (off, n) in chunks:
                    pa[off] = ppsum.tile([128, 512], f32, tag="cps", name="cps")
                    pg[off] = ppsum.tile([128, 512], f32, tag="cps", name="cps")
                last_mm = (2, 1)
                for k in range(3):
                    for icb in range(2):
                        st = (k == 0 and icb == 0)
                        sp = ((k, icb) == last_mm and bar_t is None)
                        wa = wt[:, k, icb, 128 * oco:128 * oco + 128]
                        wg = wt[:, k, icb, 256 + 128 * oco:384 + 128 * oco]
                        for (off, n) in chunks:
                            rhs = yms[(icb, off)][:, k:k + n]
                            nc.tensor.matmul(pa[off][:, :n], wa, rhs, start=st, stop=sp)
                            nc.tensor.matmul(pg[off][:, :n], wg, rhs, start=st,
                                             stop=((k, icb) == last_mm))
                if bar_t is not None:
                    for (off, n) in chunks:
                        nc.tensor.matmul(pa[off][:, :n],
                                         bar_t[blk:blk + 1, 128 * oco:128 * oco + 128],
                                         ones_t[0:1, :n],
                                         start=False, stop=True)
                for (off, n) in chunks:
                    sig = sgpool.tile([128, 512], f32, tag="sig", name="sig")
                    bias = bg_t[:, 2 * blk + oco:2 * blk + oco + 1] if bg_t is not None else 0.0
                    nc.scalar.activation(out=sig[:, :n], in_=pg[off][:, :n],
                                         func=AF.Sigmoid, bias=bias)
                    nc.vector.tensor_mul(out=sig[:, :n], in0=pa[off][:, :n], in1=sig[:, :n])
                    nc.vector.tensor_add(out=y_tiles[oco][:, off + 1:off + 1 + n],
                                         in0=sig[:, :n],
                                         in1=yms[(oco, off)][:, 1:1 + n].bitcast(f32))

        for spair in range(SPC // 2):
            ss = [2 * spair, 2 * spair + 1]
            ys = {}
            zs = {}
            vbs = {}
            vpbs = {}
            for s in ss:
                mc = mpool.tile([MEL_D, T_MEL + 2], f32r, tag="mcm", name="mcm")
                nc.sync.dma_start(out=mc[:], in_=t_mcm[s])
                vb = vpool.tile([128, T_MEL + 2], f32, tag="vm", name="vm")
                nc.gpsimd.dma_start(out=vb[:], in_=bcast(t_vm[s], 128))
                vbs[s] = vb
                yt = [ypool.tile([128, T_MEL + 2], f32r, tag="y", name="y") for _ in range(2)]
                ys[s] = yt
                for icb in range(2):
                    nc.vector.memset(yt[icb][:, 0:1].bitcast(f32), 0.0)
                    # cols beyond the GLU-written range must be exact zeros
                    nc.vector.memset(yt[icb][:, 1 + W[s]:T_MEL + 2].bitcast(f32), 0.0)
                zt = [zpool.tile([128, SP2], f32r, tag="zph", name="zph") for _ in range(2)]
                zs[s] = zt
                for icb in range(2):
                    nc.sync.dma_start(out=zt[icb][:], in_=t_zph[s, icb])
                vpb = vppool.tile([128, SP2], f32, tag="vph", name="vph")
                nc.gpsimd.dma_start(out=vpb[:], in_=bcast(t_vph[s], 128))
                vpbs[s] = vpb
                for ocb in range(2):
                    for (off, n) in mel_chunks[s]:
                        pi = ppsum.tile([128, 512], f32, tag="cps", name="cps")
                        for k in range(3):
                            nc.tensor.matmul(
                                pi[:, :n],
                                w0_t[:, k, 128 * ocb:128 * ocb + 128],
                                mc[:, off + k:off + k + n],
                                start=(k == 0), stop=(k == 2 and not has_b0))
                        if has_b0:
                            nc.tensor.matmul(pi[:, :n],
                                             b0_t[0:1, 128 * ocb:128 * ocb + 128],
                                             ones_t[0:1, :n],
                                             start=False, stop=True)
                        nc.any.tensor_copy(out=yt[ocb][:, off + 1:off + 1 + n],
                                           in_=pi[:, :n])

            for blk in range(4):
                wt = wblk.tile([128, 3, 2, 512], f32r, tag="wblk", name="wblk")
                nc.sync.dma_start(out=wt[:], in_=t_wm[blk].rearrange("k c i o -> i k c o"))
                for s in ss:
                    glu_block(ys[s], "ym", ympool, 502, mel_chunks[s], wt,
                              bmar_t if has_bm else None,
                              bmg_t if has_bm else None, blk, vbs[s])
                wtp = wblk.tile([128, 3, 2, 512], f32r, tag="wblk", name="wblk")
                nc.sync.dma_start(out=wtp[:], in_=t_wp[blk].rearrange("k c i o -> i k c o"))
                for s in ss:
                    glu_block(zs[s], "ymp", ymppool, SP2, ph_chunks[s], wtp,
                              bpar_t if has_bp else None,
                              bpg_t if has_bp else None, blk, vpbs[s])

            for s in ss:
                yt = ys[s]
                zt = zs[s]
                n_sb = NSB[s]
                mv_t = spool.tile([128, n_sb], f32, tag="mv", name="mv")
                src = t_mv[s]
                nc.gpsimd.dma_start(out=mv_t[:], in_=bass.AP(
                    tensor=src.tensor, offset=src.offset,
                    ap=[[1, 128], [128, n_sb]]))
                zts = []
                biases = []
                for sb in range(n_sb):
                    z = ztpool.tile([128, 260], f32r, tag="ztm", name="ztm")
                    sq = sqpool.tile([128, 256], f32, tag="sq", name="sq")
                    for dcb in range(2):
                        tp = tpsum.tile([128, 128], f32r, tag="tp", name="tp")
                        nc.tensor.transpose(tp[:], zt[dcb][:, 1 + 128 * sb:129 + 128 * sb], id_t[:])
                        nc.any.tensor_copy(out=z[:, 128 * dcb:128 * dcb + 128],
                                           in_=tp[:].bitcast(f32))
                        nc.scalar.square(out=sq[:, 128 * dcb:128 * dcb + 128],
                                         in_=tp[:].bitcast(f32))
                    nc.sync.dma_start(out=z[:, 256:260], in_=t_one[:])
                    ph2 = spool.tile([128, 1], f32, tag="phsq", name="phsq")
                    nc.vector.tensor_reduce(out=ph2[:], in_=sq[:], axis=AX.X, op=ALU.add)
                    bias_sb = spool.tile([128, 1], f32, tag="bias", name="bias")
                    nc.vector.tensor_scalar(out=bias_sb[:], in0=ph2[:],
                                            scalar1=-C8, scalar2=mv_t[:, sb:sb + 1],
                                            op0=ALU.mult, op1=ALU.add)
                    zts.append(z)
                    biases.append(bias_sb)
                ets = []
                for sb in range(n_sb):
                    et = epool.tile([128, T_MEL], f32r, tag="exp", name="exp")
                    for (off, n) in dot_chunks[s]:
                        dp = apsum.tile([128, 512], f32, tag="aps", name="aps")
                        for dcb in range(2):
                            nc.tensor.matmul(
                                dp[:, :n],
                                zt[dcb][:, 1 + 128 * sb:129 + 128 * sb],
                                yt[dcb][:, 1 + off:1 + off + n],
                                start=(dcb == 0), stop=(dcb == 1))
                        nc.scalar.activation(out=et[:, off:off + n], in_=dp[:, :n],
                                             func=AF.Exp, bias=biases[sb], scale=2 * C8)
                    ets.append(et)
                pad_tt = (L[s] + 1) // 128
                pad_r = (L[s] + 1) % 128
                for tt in range((Tb[s] + 127) // 128):
                    rows = min(128, Tb[s] - 128 * tt)
                    cp = cpsum.tile([128, 260], f32, tag="cxs", name="cxs")
                    for sb in range(n_sb):
                        nc.tensor.matmul(cp[:rows, :],
                                         ets[sb][:, 128 * tt:128 * tt + rows],
                                         zts[sb][:],
                                         start=(sb == 0), stop=(sb == n_sb - 1))
                    rc = spool.tile([128, 1], f32, tag="rc", name="rc")
                    nc.vector.reciprocal(out=rc[:rows], in_=cp[:rows, 256:257])
                    oc = opool.tile([128, 512], f32, tag="oc", name="oc")
                    nc.vector.tensor_scalar(out=oc[:rows, 256:512],
                                            in0=cp[:rows, 0:256],
                                            scalar1=rc[:rows], scalar2=C4,
                                            op0=ALU.mult, op1=ALU.mult)
                    for dcb in range(2):
                        tp = tpsum.tile([128, 128], f32r, tag="tp", name="tp")
                        nc.tensor.transpose(tp[:rows, :],
                                            yt[dcb][:, 1 + 128 * tt:1 + 128 * tt + rows],
                                            id_t[:])
                        nc.any.tensor_scalar_mul(out=oc[:rows, 128 * dcb:128 * dcb + 128],
                                                 in0=tp[:rows, :].bitcast(f32), scalar1=C4)
                    nc.sync.dma_start(out=t_out[s, 128 * tt:128 * tt + rows, :],
                                      in_=oc[:rows, :])
                    if tt == pad_tt and Tb[s] < T_MEL:
                        # row L[s]+1 is a padded row for every core in this slot
                        nc.sync.dma_start(out=t_pad[s], in_=oc[pad_r:pad_r + 1, :])
                        nrows = T_MEL - Tb[s]
                        pr = t_pad[s]
                        nc.sync.dma_start(
                            out=t_out[s, Tb[s]:T_MEL, :],
                            in_=bass.AP(tensor=pr.tensor, offset=pr.offset,
                                        ap=[[0, nrows], [1, 512]]))

    if not nc.is_finalized():
        nc.finalize()
    return nc


def _get_program(S_pad, L, SL, has_b0, has_bm, has_bp):
    key = (S_pad, L, SL, has_b0, has_bm, has_bp)
    if key not in _prog_cache:
        _prog_cache[key] = _build_program(S_pad, L, SL, has_b0, has_bm, has_bp)
    return _prog_cache[key]


LAST_RESULTS = None


def _install_ntff_hook():
    """Provide antenv.axon_hooks (missing in this image) so trace=True works."""
    import sys
    import types
    import ctypes
    import contextlib
    if "antenv.axon_hooks" in sys.modules:
        return
    try:
        import antenv
    except ImportError:
        return
    mod = types.ModuleType("antenv.axon_hooks")
    state = {}
    mod.set_axon_ntff_profile_hook = lambda h: state.__setitem__("h", h)
    mod.get_axon_ntff_profile_hook = lambda: state.get("h")
    sys.modules["antenv.axon_hooks"] = mod
    antenv.axon_hooks = mod
    so_path = "/opt/axon/libaxon_pjrt.so"
    if not os.path.exists(so_path):
        return
    lib = ctypes.CDLL(so_path)
    if not hasattr(lib, "axon_start_nrt_profile"):
        return
    lib.axon_start_nrt_profile.argtypes = [ctypes.POINTER(ctypes.c_int64),
                                           ctypes.c_size_t]
    lib.axon_start_nrt_profile.restype = ctypes.c_int64
    lib.axon_stop_nrt_profile.argtypes = [ctypes.c_char_p]
    lib.axon_stop_nrt_profile.restype = ctypes.c_int64

    @contextlib.contextmanager
    def _hook(output_dir, device_ids):
        import jax
        jax.devices()
        if device_ids:
            ids = (ctypes.c_int64 * len(device_ids))(*device_ids)
            rc = lib.axon_start_nrt_profile(ids, len(device_ids))
        else:
            rc = lib.axon_start_nrt_profile(None, 0)
        if rc != 0:
            raise RuntimeError(f"axon_start_nrt_profile rc={rc}")
        try:
            yield
        finally:
            n = lib.axon_stop_nrt_profile(str(output_dir).encode())
            print(f"ntff profile: {n} file(s) -> {output_dir}")

    mod.set_axon_ntff_profile_hook(_hook)


def kernel(mels, phonemes, mel_lens, phoneme_lens, embedding,
           mel_conv_w, mel_conv_b, ph_w, ph_b, mel_w, mel_b):
    global LAST_RESULTS
    from concourse.bass_utils import run_bass_kernel_spmd

    mels = np.asarray(mels)
    assert mels.shape == (B, T_MEL, MEL_D), mels.shape
    max_pl = int(np.max(np.asarray(phoneme_lens)))
    S_pad = 512 if max_pl <= 511 else 640

    in_maps, flags, perm, L, SL = _host_prep(
        np.asarray(mels), np.asarray(phonemes), np.asarray(mel_lens),
        np.asarray(phoneme_lens), np.asarray(embedding),
        np.asarray(mel_conv_w), np.asarray(mel_conv_b),
        np.asarray(ph_w), np.asarray(ph_b),
        np.asarray(mel_w), np.asarray(mel_b), S_pad)

    nc = _get_program(S_pad, L, SL, *flags)
    trace = bool(int(os.environ.get("KERNEL_TRACE", "0")))
    if trace:
        _install_ntff_hook()
    res = run_bass_kernel_spmd(nc, in_maps, core_ids=list(range(N_CORES)),
                               trace=trace,
                               tmpdir=os.environ.get("KERNEL_TRACE_DIR"))
    LAST_RESULTS = res
    out = np.empty((B, T_MEL, 512), np.float32)
    for c in range(N_CORES):
        for j in range(SPC):
            out[int(perm[8 * j + c])] = res.results[c]["out"][j]
    return out


# revision 15
# speedup vs baseline: 1.0247x; 1.0247x over previous
"""Trainium2 Bass kernel for nn_AligningModel (mel/phoneme GLU encoders + soft attention).

Strategy:
  - Data-parallel over batch: 32 samples -> 8 cores x 4 slots, length-sorted so
    each slot's compile-time bound is tight (slot j holds sorted ranks 8j..8j+7).
  - Length specialization: loop bounds per slot come from the actual mel/phoneme
    lens (program is SPMD-shared, bounds are per-slot maxima).  Valid masking
    makes everything beyond a sample's len exactly zero, so rows between len and
    the slot bound come out correct automatically; rows beyond the slot bound are
    a broadcast of the shared padded-row value.
  - Channel-major layout [C,T] on-chip so the k=3 convs are plain matmuls.
  - float32r matmuls (full PE speed at N>=256); fp32r = fp32 rounded to 11
    mantissa bits, so bytes remain valid fp32; host pre-rounds DMA-fed operands.
  - Scale folding: sqrt(0.5)^b folded into g-path conv weights; softmax uses
    logits = 2*C^8*dots(z_mel, z_ph) - ph_sq (mel_sq dropped: softmax-invariant),
    no max-subtraction (|logits| < 1 for 0.02-scaled weights), and the phoneme
    -1e9 mask folded into the per-partition exp bias.
  - Z (softmax denominator) via ones-columns appended to the time-major ph
    encoding inside the context matmul.
"""

import os
import numpy as np

B = 32
N_CORES = 8
SPC = 4           # samples (slots) per core
T_MEL = 2000
MEL_D = 80
D = 256
C = float(np.sqrt(0.5))
C4 = 0.25         # C**4 exact
C8 = 0.0625       # C**8 exact

_prog_cache = {}


def _round_fp32r(a):
    """Round fp32 to the fp32r grid (11-bit mantissa, low 12 bits zero, RNE)."""
    u = np.ascontiguousarray(a, np.float32).view(np.uint32)
    base = u >> np.uint32(12)
    low = u & np.uint32(0xFFF)
    inc = (low > 0x800) | ((low == 0x800) & ((base & np.uint32(1)) == 1))
    return ((base + inc.astype(np.uint32)) << np.uint32(12)).view(np.float32)


def _chunks(total, cap):
    """Split `total` into <=cap chunks, each a multiple of 4 (fp32r dst rule).
    Prefer equal chunks >=256 (full fp32r speed); else greedy cap + remainder."""
    assert total % 4 == 0 and total > 0
    n = -(-total // cap)
    base = min(cap, ((total + n - 1) // n + 3) // 4 * 4)
    if base < 256:
        base = cap
    out = []
    off = 0
    while off < total:
        w = min(base, total - off)
        out.append((off, w))
        off += w
    return out


def _host_prep(mels, phonemes, mel_lens, phoneme_lens, embedding,
               mel_conv_w, mel_conv_b, ph_w, ph_b, mel_w, mel_b, S_pad):
    """Build the per-core input maps (numpy only). Returns (in_maps, flags,
    perm, L, SL) where perm[8*j + c] = original sample index in core c slot j."""
    f32 = np.float32
    SP2 = S_pad + 2

    order = np.argsort(-np.asarray(mel_lens), kind="stable")
    perm = np.asarray(order)
    L = tuple(int(mel_lens[perm[8 * j]]) for j in range(SPC))
    SL = tuple(int(max(phoneme_lens[perm[8 * j + c]] for c in range(8)))
               for j in range(SPC))

    w0 = _round_fp32r(np.ascontiguousarray(np.transpose(mel_conv_w, (2, 1, 0)).astype(f32)))

    def pack_w(w4):
        out = np.empty((4, 3, 2, 128, 512), f32)
        for b in range(4):
            w = np.transpose(w4[b], (2, 1, 0)).astype(f32)  # [k, i, o]
            w = w.reshape(3, 2, 128, 512)
            w[:, :, :, 256:] *= f32(C ** b)
            out[b] = w
        return out
    wm = _round_fp32r(pack_w(mel_w))
    wp = _round_fp32r(pack_w(ph_w))
    idc4 = np.eye(128, dtype=f32)

    has_b0 = bool(np.any(mel_conv_b))
    has_bm = bool(np.any(mel_b))
    has_bp = bool(np.any(ph_b))
    shared = {"w0": w0, "wm": wm, "wp": wp, "idc4": idc4,
              "onecol": np.ones((128, 4), f32)}
    if has_b0:
        shared["b0r"] = mel_conv_b.astype(f32).reshape(1, 256)
    if has_bm:
        shared["bmar"] = mel_b[:, :256].astype(f32)
        shared["bmg"] = np.ascontiguousarray(
            mel_b[:, 256:].astype(f32).reshape(4, 2, 128).transpose(2, 0, 1).reshape(128, 8))
    if has_bp:
        shared["bpar"] = ph_b[:, :256].astype(f32)
        shared["bpg"] = np.ascontiguousarray(
            ph_b[:, 256:].astype(f32).reshape(4, 2, 128).transpose(2, 0, 1).reshape(128, 8))

    ar = np.arange(T_MEL)
    ars = np.arange(S_pad)
    in_maps = []
    for c in range(N_CORES):
        idx = [int(perm[8 * j + c]) for j in range(SPC)]
        m = dict(shared)
        mcm = np.zeros((SPC, MEL_D, T_MEL + 2), f32)
        vm = np.zeros((SPC, T_MEL + 2), f32)
        zph = np.zeros((SPC, 2, 128, SP2), f32)
        vph = np.zeros((SPC, SP2), f32)
        mv = np.full((SPC, S_pad), -1e9, f32)
        for j, b in enumerate(idx):
            mcm[j, :, 1:T_MEL + 1] = _round_fp32r(np.asarray(mels[b], f32).T)
            vm[j, 1:T_MEL + 1] = (ar < int(mel_lens[b])).astype(f32)
            pl = int(phoneme_lens[b])
            ph_pad = np.concatenate([[0], np.asarray(phonemes[b], np.int64)])[:S_pad]
            e = embedding[ph_pad].astype(f32)
            valid = (ars[:len(e)] <= pl)
            e[~valid] = 0.0
            zph[j, :, :, 1:1 + len(e)] = _round_fp32r(e.T.reshape(2, 128, len(e)))
            vph[j, 1:1 + len(e)] = valid.astype(f32)
            mv[j, :len(e)][valid] = 0.0
        m["mels_cm"] = mcm
        m["valid_mel"] = vm
        m["zph0"] = zph
        m["valid_ph"] = vph
        m["mvec"] = mv
        in_maps.append(m)
    return in_maps, (has_b0, has_bm, has_bp), perm, L, SL


def _build_program(S_pad, L, SL, has_b0, has_bm, has_bp):
    from contextlib import ExitStack
    import concourse.bass as bass
    import concourse.bacc as bacc
    import concourse.tile as tile
    from concourse import mybir

    f32 = mybir.dt.float32
    f32r = mybir.dt.float32r
    AF = mybir.ActivationFunctionType
    ALU = mybir.AluOpType
    AX = mybir.AxisListType
    SP2 = S_pad + 2

    # per-slot compile-time bounds
    W = [min(T_MEL, -(-(L[j] + 2) // 4) * 4) for j in range(SPC)]       # mel conv cols
    Tb = [min(T_MEL, -(-(L[j] + 2) // 128) * 128) for j in range(SPC)]  # attn rows
    SW = [min(S_pad, -(-(SL[j] + 2) // 4) * 4) for j in range(SPC)]     # ph conv cols
    NSB = [min(S_pad // 128, -(-(SL[j] + 2) // 128)) for j in range(SPC)]
    mel_chunks = [_chunks(W[j], 500) for j in range(SPC)]
    ph_chunks = [_chunks(SW[j], 512) for j in range(SPC)]
    dot_chunks = [_chunks(Tb[j], 512) for j in range(SPC)]

    nc = bacc.Bacc()
    t_mcm = nc.dram_tensor("mels_cm", [SPC, MEL_D, T_MEL + 2], f32r, kind="ExternalInput")
    t_vm = nc.dram_tensor("valid_mel", [SPC, T_MEL + 2], f32, kind="ExternalInput")
    t_zph = nc.dram_tensor("zph0", [SPC, 2, 128, SP2], f32r, kind="ExternalInput")
    t_vph = nc.dram_tensor("valid_ph", [SPC, SP2], f32, kind="ExternalInput")
    t_mv = nc.dram_tensor("mvec", [SPC, S_pad], f32, kind="ExternalInput")
    t_w0 = nc.dram_tensor("w0", [3, MEL_D, 256], f32r, kind="ExternalInput")
    t_wm = nc.dram_tensor("wm", [4, 3, 2, 128, 512], f32r, kind="ExternalInput")
    t_wp = nc.dram_tensor("wp", [4, 3, 2, 128, 512], f32r, kind="ExternalInput")
    t_id = nc.dram_tensor("idc4", [128, 128], f32r, kind="ExternalInput")
    t_one = nc.dram_tensor("onecol", [128, 4], f32r, kind="ExternalInput")
    t_b0 = nc.dram_tensor("b0r", [1, 256], f32, kind="ExternalInput") if has_b0 else None
    t_bmar = nc.dram_tensor("bmar", [4, 256], f32, kind="ExternalInput") if has_bm else None
    t_bmg = nc.dram_tensor("bmg", [128, 8], f32, kind="ExternalInput") if has_bm else None
    t_bpar = nc.dram_tensor("bpar", [4, 256], f32, kind="ExternalInput") if has_bp else None
    t_bpg = nc.dram_tensor("bpg", [128, 8], f32, kind="ExternalInput") if has_bp else None
    t_pad = nc.dram_tensor("padrow", [SPC, 512], f32)
    t_out = nc.dram_tensor("out", [SPC, T_MEL, 512], f32, kind="ExternalOutput")

    def bcast(ap, parts):
        return bass.AP(tensor=ap.tensor, offset=ap.offset, ap=[[0, parts]] + list(ap.ap))

    with tile.TileContext(nc) as tc, ExitStack() as ctx:
        wconst = ctx.enter_context(tc.tile_pool(name="wconst", bufs=1))
        wblk = ctx.enter_context(tc.tile_pool(name="wblk", bufs=2))
        ypool = ctx.enter_context(tc.tile_pool(name="y", bufs=4))
        zpool = ctx.enter_context(tc.tile_pool(name="zph", bufs=4))
        vpool = ctx.enter_context(tc.tile_pool(name="vm", bufs=2))
        vppool = ctx.enter_context(tc.tile_pool(name="vph", bufs=2))
        mpool = ctx.enter_context(tc.tile_pool(name="mcm", bufs=2))
        ympool = ctx.enter_context(tc.tile_pool(name="ym", bufs=10))
        ymppool = ctx.enter_context(tc.tile_pool(name="ymp", bufs=4))
        sgpool = ctx.enter_context(tc.tile_pool(name="sig", bufs=6))
        epool = ctx.enter_context(tc.tile_pool(name="exp", bufs=S_pad // 128))
        ztpool = ctx.enter_context(tc.tile_pool(name="ztm", bufs=S_pad // 128 + 2))
        sqpool = ctx.enter_context(tc.tile_pool(name="sq", bufs=2))
        spool = ctx.enter_context(tc.tile_pool(name="small", bufs=2 * (S_pad // 128) + 8))
        opool = ctx.enter_context(tc.tile_pool(name="octx", bufs=4))
        ppsum = ctx.enter_context(tc.tile_pool(name="pconv", bufs=4, space="PSUM"))
        apsum = ctx.enter_context(tc.tile_pool(name="pattn", bufs=2, space="PSUM"))
        cpsum = ctx.enter_context(tc.tile_pool(name="pctx", bufs=1, space="PSUM"))
        tpsum = ctx.enter_context(tc.tile_pool(name="ptp", bufs=1, space="PSUM"))

        # ---- constants ----
        w0_t = wconst.tile([MEL_D, 3, 256], f32r, tag="w0")
        nc.sync.dma_start(out=w0_t[:], in_=t_w0[:].rearrange("k i o -> i k o"))
        id_t = wconst.tile([128, 128], f32r, tag="id")
        nc.gpsimd.dma_start(out=id_t[:], in_=t_id[:])
        need_ones = has_b0 or has_bm or has_bp
        if need_ones:
            ones_t = wconst.tile([1, 512], f32, tag="ones")
            nc.vector.memset(ones_t[:], 1.0)
        if has_b0:
            b0_t = wconst.tile([1, 256], f32, tag="b0")
            nc.sync.dma_start(out=b0_t[:], in_=t_b0[:])
        if has_bm:
            bmar_t = wconst.tile([4, 256], f32, tag="bmar")
            nc.sync.dma_start(out=bmar_t[:], in_=t_bmar[:])
            bmg_t = wconst.tile([128, 8], f32, tag="bmg")
            nc.sync.dma_start(out=bmg_t[:], in_=t_bmg[:])
        if has_bp:
            bpar_t = wconst.tile([4, 256], f32, tag="bpar")
            nc.sync.dma_start(out=bpar_t[:], in_=t_bpar[:])
            bpg_t = wconst.tile([128, 8], f32, tag="bpg")
            nc.sync.dma_start(out=bpg_t[:], in_=t_bpg[:])

        def glu_block(y_tiles, ym_tag, ym_pool, width, chunks, wt,
                      bar_t, bg_t, blk, vb):
            """One GLU block, channel-major, in-place on y_tiles."""
            yms = {}
            for icb in range(2):
                for (off, n) in chunks:
                    ym = ym_pool.tile([128, width], f32r, tag=ym_tag, name=ym_tag)
                    nc.vector.tensor_mul(out=ym[:, :n + 2],
                                         in0=y_tiles[icb][:, off:off + n + 2].bitcast(f32),
                                         in1=vb[:, off:off + n + 2])
                    yms[(icb, off)] = ym
            for oco in range(2):
                pa = {}
                pg = {}
                for (off, n) in chunks:
                    pa[off] = ppsum.tile([128, 512], f32, tag="cps", name="cps")
                    pg[off] = ppsum.tile([128, 512], f32, tag="cps", name="cps")
                last_mm = (2, 1)
                for k in range(3):
                    for icb in range(2):
                        st = (k == 0 and icb == 0)
                        sp = ((k, icb) == last_mm and bar_t is None)
                        wa = wt[:, k, icb, 128 * oco:128 * oco + 128]
                        wg = wt[:, k, icb, 256 + 128 * oco:384 + 128 * oco]
                        for (off, n) in chunks:
                            rhs = yms[(icb, off)][:, k:k + n]
                            nc.tensor.matmul(pa[off][:, :n], wa, rhs, start=st, stop=sp)
                            nc.tensor.matmul(pg[off][:, :n], wg, rhs, start=st,
                                             stop=((k, icb) == last_mm))
                if bar_t is not None:
                    for (off, n) in chunks:
                        nc.tensor.matmul(pa[off][:, :n],
                                         bar_t[blk:blk + 1, 128 * oco:128 * oco + 128],
                                         ones_t[0:1, :n],
                                         start=False, stop=True)
                for (off, n) in chunks:
                    sig = sgpool.tile([128, 512], f32, tag="sig", name="sig")
                    bias = bg_t[:, 2 * blk + oco:2 * blk + oco + 1] if bg_t is not None else 0.0
                    nc.scalar.activation(out=sig[:, :n], in_=pg[off][:, :n],
                                         func=AF.Sigmoid, bias=bias)
                    nc.vector.tensor_mul(out=sig[:, :n], in0=pa[off][:, :n], in1=sig[:, :n])
                    nc.vector.tensor_add(out=y_tiles[oco][:, off + 1:off + 1 + n],
                                         in0=sig[:, :n],
                                         in1=yms[(oco, off)][:, 1:1 + n].bitcast(f32))

        for ss in ([2, 1], [3, 0]):
            ys = {}
            zs = {}
            vbs = {}
            vpbs = {}
            for s in ss:
                mc = mpool.tile([MEL_D, T_MEL + 2], f32r, tag="mcm", name="mcm")
                nc.sync.dma_start(out=mc[:], in_=t_mcm[s])
                vb = vpool.tile([128, T_MEL + 2], f32, tag="vm", name="vm")
                nc.gpsimd.dma_start(out=vb[:], in_=bcast(t_vm[s], 128))
                vbs[s] = vb
                yt = [ypool.tile([128, T_MEL + 2], f32r, tag="y", name="y") for _ in range(2)]
                ys[s] = yt
                for icb in range(2):
                    nc.vector.memset(yt[icb][:, 0:1].bitcast(f32), 0.0)
                    # cols beyond the GLU-written range must be exact zeros
                    nc.vector.memset(yt[icb][:, 1 + W[s]:T_MEL + 2].bitcast(f32), 0.0)
                zt = [zpool.tile([128, SP2], f32r, tag="zph", name="zph") for _ in range(2)]
                zs[s] = zt
                for icb in range(2):
                    nc.gpsimd.dma_start(out=zt[icb][:], in_=t_zph[s, icb])
                vpb = vppool.tile([128, SP2], f32, tag="vph", name="vph")
                nc.gpsimd.dma_start(out=vpb[:], in_=bcast(t_vph[s], 128))
                vpbs[s] = vpb
                for ocb in range(2):
                    for (off, n) in mel_chunks[s]:
                        pi = ppsum.tile([128, 512], f32, tag="cps", name="cps")
                        for k in range(3):
                            nc.tensor.matmul(
                                pi[:, :n],
                                w0_t[:, k, 128 * ocb:128 * ocb + 128],
                                mc[:, off + k:off + k + n],
                                start=(k == 0), stop=(k == 2 and not has_b0))
                        if has_b0:
                            nc.tensor.matmul(pi[:, :n],
                                             b0_t[0:1, 128 * ocb:128 * ocb + 128],
                                             ones_t[0:1, :n],
                                             start=False, stop=True)
                        nc.any.tensor_copy(out=yt[ocb][:, off + 1:off + 1 + n],
                                           in_=pi[:, :n])

            for blk in range(4):
                wt = wblk.tile([128, 3, 2, 512], f32r, tag="wblk", name="wblk")
                nc.gpsimd.dma_start(out=wt[:], in_=t_wm[blk].rearrange("k c i o -> i k c o"))
                for s in ss:
                    glu_block(ys[s], "ym", ympool, 502, mel_chunks[s], wt,
                              bmar_t if has_bm else None,
                              bmg_t if has_bm else None, blk, vbs[s])
                wtp = wblk.tile([128, 3, 2, 512], f32r, tag="wblk", name="wblk")
                nc.gpsimd.dma_start(out=wtp[:], in_=t_wp[blk].rearrange("k c i o -> i k c o"))
                for s in ss:
                    glu_block(zs[s], "ymp", ymppool, SP2, ph_chunks[s], wtp,
                              bpar_t if has_bp else None,
                              bpg_t if has_bp else None, blk, vpbs[s])

            for s in ss:
                yt = ys[s]
                zt = zs[s]
                n_sb = NSB[s]
                mv_t = spool.tile([128, n_sb], f32, tag="mv", name="mv")
                src = t_mv[s]
                nc.gpsimd.dma_start(out=mv_t[:], in_=bass.AP(
                    tensor=src.tensor, offset=src.offset,
                    ap=[[1, 128], [128, n_sb]]))
                zts = []
                biases = []
                for sb in range(n_sb):
                    z = ztpool.tile([128, 260], f32r, tag="ztm", name="ztm")
                    sq = sqpool.tile([128, 256], f32, tag="sq", name="sq")
                    for dcb in range(2):
                        tp = tpsum.tile([128, 128], f32r, tag="tp", name="tp")
                        nc.tensor.transpose(tp[:], zt[dcb][:, 1 + 128 * sb:129 + 128 * sb], id_t[:])
                        nc.any.tensor_copy(out=z[:, 128 * dcb:128 * dcb + 128],
                                           in_=tp[:].bitcast(f32))
                        nc.scalar.square(out=sq[:, 128 * dcb:128 * dcb + 128],
                                         in_=tp[:].bitcast(f32))
                    nc.sync.dma_start(out=z[:, 256:260], in_=t_one[:])
                    ph2 = spool.tile([128, 1], f32, tag="phsq", name="phsq")
                    nc.vector.tensor_reduce(out=ph2[:], in_=sq[:], axis=AX.X, op=ALU.add)
                    bias_sb = spool.tile([128, 1], f32, tag="bias", name="bias")
                    nc.vector.tensor_scalar(out=bias_sb[:], in0=ph2[:],
                                            scalar1=-C8, scalar2=mv_t[:, sb:sb + 1],
                                            op0=ALU.mult, op1=ALU.add)
                    zts.append(z)
                    biases.append(bias_sb)
                ets = []
                for sb in range(n_sb):
                    et = epool.tile([128, T_MEL], f32r, tag="exp", name="exp")
                    for (off, n) in dot_chunks[s]:
                        dp = apsum.tile([128, 512], f32, tag="aps", name="aps")
                        for dcb in range(2):
                            nc.tensor.matmul(
                                dp[:, :n],
                                zt[dcb][:, 1 + 128 * sb:129 + 128 * sb],
                                yt[dcb][:, 1 + off:1 + off + n],
                                start=(dcb == 0), stop=(dcb == 1))
                        nc.scalar.activation(out=et[:, off:off + n], in_=dp[:, :n],
                                             func=AF.Exp, bias=biases[sb], scale=2 * C8)
                    ets.append(et)
                pad_tt = (L[s] + 1) // 128
                pad_r = (L[s] + 1) % 128
                for tt in range((Tb[s] + 127) // 128):
                    rows = min(128, Tb[s] - 128 * tt)
                    cp = cpsum.tile([128, 260], f32, tag="cxs", name="cxs")
                    for sb in range(n_sb):
                        nc.tensor.matmul(cp[:rows, :],
                                         ets[sb][:, 128 * tt:128 * tt + rows],
                                         zts[sb][:],
                                         start=(sb == 0), stop=(sb == n_sb - 1))
                    rc = spool.tile([128, 1], f32, tag="rc", name="rc")
                    nc.vector.reciprocal(out=rc[:rows], in_=cp[:rows, 256:257])
                    oc = opool.tile([128, 512], f32, tag="oc", name="oc")
                    nc.vector.tensor_scalar(out=oc[:rows, 256:512],
                                            in0=cp[:rows, 0:256],
                                            scalar1=rc[:rows], scalar2=C4,
                                            op0=ALU.mult, op1=ALU.mult)
                    for dcb in range(2):
                        tp = tpsum.tile([128, 128], f32r, tag="tp", name="tp")
                        nc.tensor.transpose(tp[:rows, :],
                                            yt[dcb][:, 1 + 128 * tt:1 + 128 * tt + rows],
                                            id_t[:])
                        nc.any.tensor_scalar_mul(out=oc[:rows, 128 * dcb:128 * dcb + 128],
                                                 in0=tp[:rows, :].bitcast(f32), scalar1=C4)
                    nc.sync.dma_start(out=t_out[s, 128 * tt:128 * tt + rows, :],
                                      in_=oc[:rows, :])
                    if tt == pad_tt and Tb[s] < T_MEL:
                        # row L[s]+1 is a padded row for every core in this slot
                        nc.sync.dma_start(out=t_pad[s], in_=oc[pad_r:pad_r + 1, :])
                        nrows = T_MEL - Tb[s]
                        pr = t_pad[s]
                        nc.sync.dma_start(
                            out=t_out[s, Tb[s]:T_MEL, :],
                            in_=bass.AP(tensor=pr.tensor, offset=pr.offset,
                                        ap=[[0, nrows], [1, 512]]))

    if not nc.is_finalized():
        nc.finalize()
    return nc


def _get_program(S_pad, L, SL, has_b0, has_bm, has_bp):
    key = (S_pad, L, SL, has_b0, has_bm, has_bp)
    if key not in _prog_cache:
        _prog_cache[key] = _build_program(S_pad, L, SL, has_b0, has_bm, has_bp)
    return _prog_cache[key]


LAST_RESULTS = None


def _install_ntff_hook():
    """Provide antenv.axon_hooks (missing in this image) so trace=True works."""
    import sys
    import types
    import ctypes
    import contextlib
    if "antenv.axon_hooks" in sys.modules:
        return
    try:
        import antenv
    except ImportError:
        return
    mod = types.ModuleType("antenv.axon_hooks")
    state = {}
    mod.set_axon_ntff_profile_hook = lambda h: state.__setitem__("h", h)
    mod.get_axon_ntff_profile_hook = lambda: state.get("h")
    sys.modules["antenv.axon_hooks"] = mod
    antenv.axon_hooks = mod
    so_path = "/opt/axon/libaxon_pjrt.so"
    if not os.path.exists(so_path):
        return
    lib = ctypes.CDLL(so_path)
    if not hasattr(lib, "axon_start_nrt_profile"):
        return
    lib.axon_start_nrt_profile.argtypes = [ctypes.POINTER(ctypes.c_int64),
                                           ctypes.c_size_t]
    lib.axon_start_nrt_profile.restype = ctypes.c_int64
    lib.axon_stop_nrt_profile.argtypes = [ctypes.c_char_p]
    lib.axon_stop_nrt_profile.restype = ctypes.c_int64

    @contextlib.contextmanager
    def _hook(output_dir, device_ids):
        import jax
        jax.devices()
        if device_ids:
            ids = (ctypes.c_int64 * len(device_ids))(*device_ids)
            rc = lib.axon_start_nrt_profile(ids, len(device_ids))
        else:
            rc = lib.axon_start_nrt_profile(None, 0)
        if rc != 0:
            raise RuntimeError(f"axon_start_nrt_profile rc={rc}")
        try:
            yield
        finally:
            n = lib.axon_stop_nrt_profile(str(output_dir).encode())
            print(f"ntff profile: {n} file(s) -> {output_dir}")

    mod.set_axon_ntff_profile_hook(_hook)


def kernel(mels, phonemes, mel_lens, phoneme_lens, embedding,
           mel_conv_w, mel_conv_b, ph_w, ph_b, mel_w, mel_b):
    global LAST_RESULTS
    from concourse.bass_utils import run_bass_kernel_spmd

    mels = np.asarray(mels)
    assert mels.shape == (B, T_MEL, MEL_D), mels.shape
    max_pl = int(np.max(np.asarray(phoneme_lens)))
    S_pad = 512 if max_pl <= 511 else 640

    in_maps, flags, perm, L, SL = _host_prep(
        np.asarray(mels), np.asarray(phonemes), np.asarray(mel_lens),
        np.asarray(phoneme_lens), np.asarray(embedding),
        np.asarray(mel_conv_w), np.asarray(mel_conv_b),
        np.asarray(ph_w), np.asarray(ph_b),
        np.asarray(mel_w), np.asarray(mel_b), S_pad)

    nc = _get_program(S_pad, L, SL, *flags)
    trace = bool(int(os.environ.get("KERNEL_TRACE", "0")))
    if trace:
        _install_ntff_hook()
    res = run_bass_kernel_spmd(nc, in_maps, core_ids=list(range(N_CORES)),
                               trace=trace,
                               tmpdir=os.environ.get("KERNEL_TRACE_DIR"))
    LAST_RESULTS = res
    out = np.empty((B, T_MEL, 512), np.float32)
    for c in range(N_CORES):
        for j in range(SPC):
            out[int(perm[8 * j + c])] = res.results[c]["out"][j]
    return out
